# revision 1
# baseline (speedup 1.0000x reference)
"""CrossAttentionHead kernel for 8x TRN2 NeuronCores (Bass/Tile), v2.

Reference (fp32):
    Q = q @ Wq.T + bq          # [S, DQ]      S=4096, DQ=1024
    K = k @ Wk.T + bk          # [S, DK]      DK=4096
    V = v @ Wv.T + bv          # [S, DK]
    numT = K.T @ Q             # [DK, DQ]
    attn = softmax(num / 64, axis over DK)
    out  = attn @ V            # [DQ, DK]

Sharding: tensor-parallel over dim_k; core i owns k-columns
[i*512, (i+1)*512).  Restructure kills the replicated Q projection:
    K_i  = k @ Wk_i^T + bk_i                 # [S, 512]
    A_i  = q^T K_i                           # [DIN, 512]  (contraction S)
    numT = A_i^T-as-lhsT @ Wq^T + colsum(K_i) x bq   # [512, DQ]
    E_i  = exp(numT / 64);  s_i = colsum(E_i)
    P_i  = E_i^T @ (v_i @ Wv^T)              # [DQ, DK] partial
Host merges: out = (sum P_i) / (sum s_i) + bv.

Precision: fp8e4m3 operands with DoubleRow matmuls (2 contraction
tiles / instruction).  Pure-fp8 tensors: k (1x), q (1/2 scale),
Wk (32x) and the on-chip K (32x).  Hi+lo split-fp8 (residual)
tensors: Wq,Wv (32x scale), V (32x), A (1/2), E (1x).  All psum
scales chosen so hi=copy(psum), lo=sub(psum,hi) need no rescaling
except A (tensor_scalar pass).
Scale bookkeeping:
    psum_K = 32K;  psum_A = 16A;  psum_num = 16*numT  (exp scale 1/1024)
    psum_V = 32V;  psum_P = 32P (evict fp16 w/ scale 1/32); psum_s = s
Validated vs reference data: maxrel 1.675e-2 (gate 2e-2).
"""

import threading

import numpy as np
import ml_dtypes

S = 4096
DIN = 1024
DQ = 1024
DK = 4096
NCORES = 8
KSH = DK // NCORES          # 512: per-core shard of dim_k
P = 128
ST = S // P                 # 32 s-tiles
CT = DIN // P               # 8 contraction chunks over DIN
KC = KSH // P               # 4 partition chunks of the k-shard
E_SCALE = 1.0 / 1024.0      # exp(psum_num * E_SCALE) = exp(numT/64)

_lock = threading.Lock()
_cache = {}


def _build_module():
    import concourse.bacc as bacc
    import concourse.mybir as mybir
    import concourse.tile as tile

    f8 = mybir.dt.float8e4
    f16 = mybir.dt.float16
    f32 = mybir.dt.float32
    Exp = mybir.ActivationFunctionType.Exp
    Copy = mybir.ActivationFunctionType.Copy
    DR = mybir.MatmulPerfMode.DoubleRow
    Mult = mybir.AluOpType.mult

    nc = bacc.Bacc(
        "TRN2", target_bir_lowering=False, debug=False, num_devices=NCORES
    )

    # DRAM inputs (per-core layouts prepped host-side)
    kT8g = nc.dram_tensor(
        "kT8", [ST // 8, P, 8, CT, P], f8, kind="ExternalInput").ap()
    q8 = nc.dram_tensor("q8", [P, ST, DIN], f8, kind="ExternalInput").ap()
    wkhi = nc.dram_tensor("wkhi", [P, CT, KSH], f8, kind="ExternalInput").ap()
    wqhi = nc.dram_tensor("wqhi", [P, CT, DQ], f8, kind="ExternalInput").ap()
    wqlo = nc.dram_tensor("wqlo", [P, CT, DQ], f8, kind="ExternalInput").ap()
    wvhi = nc.dram_tensor("wvhi", [P, CT, DK], f8, kind="ExternalInput").ap()
    wvlo = nc.dram_tensor("wvlo", [P, CT, DK], f8, kind="ExternalInput").ap()
    vThi = nc.dram_tensor("vThi", [P, CT, KSH], f8, kind="ExternalInput").ap()
    vTlo = nc.dram_tensor("vTlo", [P, CT, KSH], f8, kind="ExternalInput").ap()
    bk2 = nc.dram_tensor("bk2", [1, 2, KSH], f8, kind="ExternalInput").ap()
    bkb = nc.dram_tensor("bkb", [2 * KSH], f8, kind="ExternalInput").ap()
    bq16 = nc.dram_tensor(
        "bq16", [1, 2, DQ], f8, kind="ExternalInput").ap()
    p_out = nc.dram_tensor("p_out", [DQ, DK], f16, kind="ExternalOutput").ap()
    s_out = nc.dram_tensor("s_out", [1, DQ], f32, kind="ExternalOutput").ap()

    def ts(i, sz):
        return slice(i * sz, (i + 1) * sz)

    def pr(i):  # DoubleRow pair slice over a chunk axis
        return slice(2 * i, 2 * i + 2)

    with tile.TileContext(nc) as tc:
        with tc.tile_pool(name="persist", bufs=1) as persist:
            # Persistent SBUF tensors
            q_sb = persist.tile([P, ST, DIN], f8)       # 32 KB/part
            Khi_sb = persist.tile([P, ST, KSH], f8)     # 16 KB
            Ahi_sb = persist.tile([P, CT, KSH], f8)     # 4 KB
            Alo_sb = persist.tile([P, CT, KSH], f8)     # 4 KB
            Ef_sb = persist.tile([P, KC, DQ], f16)      # 8 KB
            Ehi_sb = persist.tile([P, KC, DQ], f8)      # 4 KB
            Elo_sb = persist.tile([P, KC, DQ], f8)      # 4 KB
            Vhi_sb = persist.tile([P, KC, DK], f8)      # 16 KB
            Vlo_sb = persist.tile([P, KC, DK], f8)      # 16 KB
            wkhi_sb = persist.tile([P, CT, KSH], f8)    # 4 KB
            wqhi_sb = persist.tile([P, CT, DQ], f8)     # 8 KB
            wqlo_sb = persist.tile([P, CT, DQ], f8)     # 8 KB
            vThi_sb = persist.tile([P, CT, KSH], f8)    # 4 KB
            vTlo_sb = persist.tile([P, CT, KSH], f8)    # 4 KB
            bk2_sb = persist.tile([1, 2, KSH], f8)
            bkb_sb = persist.tile([P, 2, KSH], f8)
            bq2_sb = persist.tile([1, 2, DQ], f8)
            cs2_sb = persist.tile([1, 2, KSH], f8)
            s_sb = persist.tile([1, DQ], f32)
            ones2c = persist.tile([P, 2, P], f8)        # DR ones, 128-part
            ones2r = persist.tile([1, 2, P], f8)        # DR ones, 1-part
            warm_rhs = persist.tile([1, 2, 512], f8)
            warm_out = persist.tile([P, 512], f16)
            nc.vector.memset(ones2c[:], 1.0)
            nc.vector.memset(ones2r[:], 1.0)
            nc.vector.memset(warm_rhs[:], 0.0)

            # ---- input DMA (sync queue), ordered by first use ----
            # vT first: V-proj fc=0 is the cold-start filler phase
            nc.sync.dma_start(vThi_sb[:, 0:4], vThi[:, 0:4])

            wvp = tc.alloc_tile_pool(name="wv", bufs=4)
            ktp = tc.alloc_tile_pool(name="kt", bufs=3)
            # prefetch wv chunks fc=0,1 early (scalar/ACT queue)
            wv_tiles = {}
            whi0 = wvp.tile([P, CT, 512], f8, tag="wvhi")
            nc.sync.dma_start(whi0[:, 0:4], wvhi[:, 0:4, ts(0, 512)])
            nc.sync.dma_start(vThi_sb[:, 4:8], vThi[:, 4:8])
            nc.sync.dma_start(whi0[:, 4:8], wvhi[:, 4:8, ts(0, 512)])
            wlo0 = wvp.tile([P, CT, 512], f8, tag="wvlo")
            nc.sync.dma_start(wlo0[:], wvlo[:, :, ts(0, 512)])
            wv_tiles[0] = (whi0, wlo0)
            nc.sync.dma_start(vTlo_sb[:], vTlo)
            nc.sync.dma_start(wkhi_sb[:, 0:2], wkhi[:, 0:2])
            nc.sync.dma_start(
                bkb_sb[:], bkb.unsqueeze(0).to_broadcast((P, 2 * KSH)))

            # One shared PSUM pool: 4 slots x [P, 1024] f32 = all 8 banks.
            with tc.tile_pool(name="psum", bufs=4, space="PSUM") as pp:

                # PE warmup: dummy matmuls ramp the p-state to full clock
                # while the first input DMAs are still in flight
                wps = pp.tile([P, 1024], f32, tag="ps", name="warm")
                for i in range(10):
                    nc.tensor.matmul(
                        wps[:, 0:512], ones2r[:], warm_rhs[:],
                        start=(i == 0), stop=(i == 10 - 1), perf_mode=DR,
                    )
                nc.scalar.activation(warm_out[:], wps[:, 0:512], Copy)

                def _vproj_fc(fc, prefetch=True):
                    if prefetch and fc + 3 < CT and fc + 3 not in wv_tiles:
                        whi3 = wvp.tile([P, CT, 512], f8, tag="wvhi")
                        nc.sync.dma_start(
                            whi3[:], wvhi[:, :, ts(fc + 3, 512)])
                        wlo3 = wvp.tile([P, CT, 512], f8, tag="wvlo")
                        nc.sync.dma_start(
                            wlo3[:], wvlo[:, :, ts(fc + 3, 512)])
                        wv_tiles[fc + 3] = (whi3, wlo3)
                    if prefetch and fc + 2 < CT and fc + 2 not in wv_tiles:
                        whi2 = wvp.tile([P, CT, 512], f8, tag="wvhi")
                        nc.sync.dma_start(
                            whi2[:], wvhi[:, :, ts(fc + 2, 512)])
                        wlo2 = wvp.tile([P, CT, 512], f8, tag="wvlo")
                        nc.sync.dma_start(
                            wlo2[:], wvlo[:, :, ts(fc + 2, 512)])
                        wv_tiles[fc + 2] = (whi2, wlo2)
                    whi, wlo = wv_tiles.pop(fc)
                    if fc == 0:
                        # cold start: term-major, each term gated on a
                        # single DMA arrival
                        pss = [pp.tile([P, 1024], f32, tag="ps",
                                       name=f"psv0_{c}") for c in range(2)]
                        for term in range(3):
                            for c in range(KC // 2):
                                for i in range(2):
                                    kc = 2 * c + i
                                    out = pss[c][:, ts(i, 512)]
                                    for dp in range(CT // 2):
                                        lhs = (vThi_sb if term < 2
                                               else vTlo_sb)[:, pr(dp), ts(kc, P)]
                                        rhs = (wlo if term == 1 else whi)[
                                            :, pr(dp), :]
                                        nc.tensor.matmul(
                                            out, lhs, rhs,
                                            start=(term == 0 and dp == 0),
                                            stop=(term == 2
                                                  and dp == CT // 2 - 1),
                                            perf_mode=DR)
                        for c in range(KC // 2):
                            psv = pss[c][:].rearrange("p (u n) -> p u n", u=2)
                            tgt_hi = Vhi_sb[:, pr(c), ts(fc, 512)]
                            nc.scalar.activation(tgt_hi, psv, Copy)
                            nc.vector.tensor_sub(
                                Vlo_sb[:, pr(c), ts(fc, 512)], psv, tgt_hi)
                        return
                    for c in range(KC // 2):
                        ps = pp.tile([P, 1024], f32, tag="ps")
                        for i in range(2):
                            kc = 2 * c + i
                            out = ps[:, ts(i, 512)]
                            for dp in range(CT // 2):
                                lhs_hi = vThi_sb[:, pr(dp), ts(kc, P)]
                                lhs_lo = vTlo_sb[:, pr(dp), ts(kc, P)]
                                nc.tensor.matmul(
                                    out, lhs_hi, whi[:, pr(dp), :],
                                    start=(dp == 0), stop=False, perf_mode=DR)
                                nc.tensor.matmul(
                                    out, lhs_hi, wlo[:, pr(dp), :],
                                    start=False, stop=False, perf_mode=DR)
                                nc.tensor.matmul(
                                    out, lhs_lo, whi[:, pr(dp), :],
                                    start=False, stop=(dp == CT // 2 - 1),
                                    perf_mode=DR)
                        psv = ps[:].rearrange("p (u n) -> p u n", u=2)
                        tgt_hi = Vhi_sb[:, pr(c), ts(fc, 512)]
                        nc.scalar.activation(tgt_hi, psv, Copy)
                        nc.vector.tensor_sub(
                            Vlo_sb[:, pr(c), ts(fc, 512)], psv, tgt_hi)

                # ---- phase 0: V proj fc=0 (cold-start filler) ----
                _vproj_fc(0, prefetch=False)

                # ---- phase 1: K proj (psum = 32K, 2 s-tiles per slot) ----
                kts = [None] * (ST // 8)

                def _fetch_kt(j):
                    kt = ktp.tile([P, 8, CT, P], f8, tag="kt")
                    if j == 0:
                        # interleave wk chunks with the first st tiles, in
                        # the dp-major order group 0 consumes them
                        nc.sync.dma_start(kt[:, 0:2], kT8g[0][:, 0:2])
                        nc.sync.dma_start(wkhi_sb[:, 2:4], wkhi[:, 2:4])
                        nc.sync.dma_start(kt[:, 2:4], kT8g[0][:, 2:4])
                        nc.sync.dma_start(wkhi_sb[:, 4:6], wkhi[:, 4:6])
                        nc.sync.dma_start(kt[:, 4:6], kT8g[0][:, 4:6])
                        nc.sync.dma_start(wkhi_sb[:, 6:8], wkhi[:, 6:8])
                        nc.sync.dma_start(bk2_sb[:], bk2)
                        nc.sync.dma_start(kt[:, 6:8], kT8g[0][:, 6:8])
                    else:
                        nc.sync.dma_start(kt[:], kT8g[j])
                    kts[j] = kt

                _fetch_kt(0)
                _fetch_kt(1)

                # group 0 runs dp-major: each (dp, hi/lo) is an 8-st burst
                # needing only one fresh wk chunk, so the cold-start DMA
                # latencies overlap compute instead of serializing
                kt = kts[0]
                ps0 = [pp.tile([P, 1024], f32, tag="ps", name=f"ps0_{g}")
                       for g in range(4)]
                for dp in range(CT // 2):
                    for st in range(8):
                        odd = (st // 2) % 2 == 1
                        nc.tensor.matmul(
                            ps0[st // 2][:, ts(st % 2, KSH)],
                            kt[:, st, pr(dp), :], wkhi_sb[:, pr(dp), :],
                            start=(dp == 0),
                            stop=(odd and dp == CT // 2 - 1), perf_mode=DR,
                        )
                for st in range(8):
                    g = st // 2
                    if g % 2 == 0:
                        nc.tensor.matmul(
                            ps0[g][:, ts(st % 2, KSH)], ones2r[:], bk2_sb[:],
                            start=False, stop=True, perf_mode=DR,
                        )
                    if st % 2 == 1:  # evict each slot as soon as it closes
                        psv = ps0[g][:].rearrange("p (u n) -> p u n", u=2)
                        if g % 2 == 0:
                            nc.scalar.activation(
                                Khi_sb[:, ts(g, 2)], psv, Copy)
                        else:
                            # bias folds into the DVE eviction
                            nc.vector.tensor_add(
                                Khi_sb[:, ts(g, 2)], psv, bkb_sb[:])
                _fetch_kt(2)

                for j in range(1, ST // 8):
                    if j + 2 < ST // 8:
                        _fetch_kt(j + 2)
                    nc.sync.dma_start(
                        q_sb[:, ts(2 * j - 2, 4)], q8[:, ts(2 * j - 2, 4)])
                    nc.sync.dma_start(
                        q_sb[:, ts(2 * j - 1, 4)], q8[:, ts(2 * j - 1, 4)])
                    kt = kts[j]
                    for g in range(4):
                        ps = pp.tile([P, 1024], f32, tag="ps")
                        for u in range(2):
                            st = 8 * j + 2 * g + u
                            out = ps[:, ts(u, KSH)]
                            lt = kt[:, 2 * g + u]
                            for dp in range(CT // 2):
                                nc.tensor.matmul(
                                    out, lt[:, pr(dp), :], wkhi_sb[:, pr(dp), :],
                                    start=(dp == 0),
                                    stop=(g % 2 == 1 and dp == CT // 2 - 1),
                                    perf_mode=DR,
                                )
                            if g % 2 == 0:
                                nc.tensor.matmul(
                                    out, ones2r[:], bk2_sb[:],
                                    start=False, stop=True, perf_mode=DR,
                                )
                        sl2 = ts(4 * j + g, 2)
                        psv = ps[:].rearrange("p (u n) -> p u n", u=2)
                        if g % 2 == 0:
                            nc.scalar.activation(Khi_sb[:, sl2], psv, Copy)
                        else:
                            # bias folds into the DVE eviction
                            nc.vector.tensor_add(Khi_sb[:, sl2], psv, bkb_sb[:])
                for c in range(6, 8):
                    nc.sync.dma_start(q_sb[:, ts(c, 4)], q8[:, ts(c, 4)])
                _whi1 = wvp.tile([P, CT, 512], f8, tag="wvhi")
                nc.sync.dma_start(_whi1[:], wvhi[:, :, ts(1, 512)])
                _wlo1 = wvp.tile([P, CT, 512], f8, tag="wvlo")
                nc.sync.dma_start(_wlo1[:], wvlo[:, :, ts(1, 512)])
                wv_tiles[1] = (_whi1, _wlo1)
                nc.sync.dma_start(bq2_sb[:], bq16)
                nc.sync.dma_start(wqhi_sb[:], wqhi)
                nc.sync.dma_start(wqlo_sb[:], wqlo)
                ktp.release()
                atp = tc.alloc_tile_pool(name="at", bufs=1)

                # ---- phase 2: A = q^T K (psum = 16A), 2 passes x 4 dc ----
                for pas in range(2):
                    pss = [pp.tile([P, 1024], f32, tag="ps",
                                   name=f"psa{pas}_{h}") for h in range(2)]
                    for t in range(ST // 2):
                        for i in range(4):
                            dc = 4 * pas + i
                            out = pss[i // 2][:, ts(i % 2, KSH)]
                            lhs = q_sb[:, pr(t), ts(dc, P)]
                            nc.tensor.matmul(
                                out, lhs, Khi_sb[:, pr(t), :],
                                start=(t == 0), stop=(t == ST // 2 - 1),
                                perf_mode=DR,
                            )
                    for h in range(2):
                        sl2 = ts(2 * pas + h, 2)
                        psv = pss[h][:].rearrange("p (u n) -> p u n", u=2)
                        nc.scalar.activation(
                            Ahi_sb[:, sl2], psv, Copy, scale=1.0 / 32.0)
                        at = atp.tile([P, 1024], f16, tag="at")
                        atv = at[:].rearrange("p (u n) -> p u n", u=2)
                        nc.vector.tensor_scalar(
                            atv, psv, 1.0 / 32.0, None, Mult)
                        nc.vector.tensor_sub(
                            Alo_sb[:, sl2], atv, Ahi_sb[:, sl2])
                atp.release()

                # ---- phase 3: cs = colsum(K) (psum = 32cs, evict cs/2) ----
                # (also absorbs the A-eviction latency before num starts)
                psc = pp.tile([P, KSH], f32, tag="ps", name="psc")
                for t in range(ST // 2):
                    nc.tensor.matmul(
                        psc[:], ones2c[:], Khi_sb[:, pr(t), :],
                        start=(t == 0), stop=(t == ST // 2 - 1), perf_mode=DR,
                    )
                nc.scalar.activation(
                    cs2_sb[:, 0], psc[0:1, :], Copy, scale=1.0 / 64.0)
                nc.scalar.activation(
                    cs2_sb[:, 1], psc[0:1, :], Copy, scale=1.0 / 64.0)

                # ---- phase 4a: V proj fc=0 (covers the A-evict latency
                # before num; E not needed yet) ----
                # wv fc=2..4 prefetch: transfers ride the DMA-idle window
                for _pf in (2, 3, 4):
                    _whi = wvp.tile([P, CT, 512], f8, tag="wvhi")
                    nc.sync.dma_start(_whi[:], wvhi[:, :, ts(_pf, 512)])
                    _wlo = wvp.tile([P, CT, 512], f8, tag="wvlo")
                    nc.sync.dma_start(_wlo[:], wvlo[:, :, ts(_pf, 512)])
                    wv_tiles[_pf] = (_whi, _wlo)

                # ---- phase 4a: V proj fc=1 covers the A-evict latency ----
                _vproj_fc(1, prefetch=False)

                # ---- phase 4: numT (psum = 16*numT) + exp -> E hi/lo ----
                for kc in range(KC):
                    ps = pp.tile([P, DQ], f32, tag="ps", name=f"psn{kc}")
                    for h in range(2):
                        out = ps[:, ts(h, 512)]
                        for dp in range(CT // 2):
                            lhs_hi = Ahi_sb[:, pr(dp), ts(kc, P)]
                            lhs_lo = Alo_sb[:, pr(dp), ts(kc, P)]
                            rhs_hi = wqhi_sb[:, pr(dp), ts(h, 512)]
                            rhs_lo = wqlo_sb[:, pr(dp), ts(h, 512)]
                            nc.tensor.matmul(
                                out, lhs_hi, rhs_hi, start=(dp == 0),
                                stop=False, perf_mode=DR)
                            nc.tensor.matmul(
                                out, lhs_hi, rhs_lo, start=False,
                                stop=False, perf_mode=DR)
                            nc.tensor.matmul(
                                out, lhs_lo, rhs_hi, start=False,
                                stop=False, perf_mode=DR)
                        # + 2 x (cs/2) x (16bq) = 16 * cs x bq  (DoubleRow)
                        nc.tensor.matmul(
                            out, cs2_sb[:, :, ts(kc, P)],
                            bq2_sb[:, :, ts(h, 512)],
                            start=False, stop=True, perf_mode=DR,
                        )
                    nc.scalar.activation(
                        Ef_sb[:, kc], ps[:], Exp, scale=E_SCALE)
                    nc.vector.tensor_copy(Ehi_sb[:, kc], Ef_sb[:, kc])
                    nc.vector.tensor_sub(
                        Elo_sb[:, kc], Ef_sb[:, kc], Ehi_sb[:, kc])

                # ---- phase 5/6: remaining V proj + s denominators ----
                for fc in range(2, CT):
                    _vproj_fc(fc)
                    if fc == 2:
                        # s = colsum(E); E is ready by now, PE otherwise busy
                        for h in range(2):
                            ps = pp.tile([P, 512], f32, tag="ps", name=f"pss{h}")
                            for c in range(KC // 2):
                                nc.tensor.matmul(
                                    ps[:], ones2c[:],
                                    Ehi_sb[:, pr(c), ts(h, 512)],
                                    start=(c == 0), stop=False, perf_mode=DR,
                                )
                                nc.tensor.matmul(
                                    ps[:], ones2c[:],
                                    Elo_sb[:, pr(c), ts(h, 512)],
                                    start=False, stop=(c == KC // 2 - 1),
                                    perf_mode=DR,
                                )
                            nc.vector.tensor_copy(s_sb[:, ts(h, 512)], ps[0:1, :])
                        nc.sync.dma_start(s_out, s_sb[:])
                wvp.release()

                # ---- phase 7: P = E^T V (psum = 32P -> fp16 out) ----
                with tc.tile_pool(name="ost", bufs=2) as ost:
                    for dqt in range(DQ // P):
                        ot = ost.tile([P, 4, 1024], f16, tag="ot")
                        for g in range(4):  # fc pairs
                            ps = pp.tile([P, 1024], f32, tag="ps")
                            for u in range(2):
                                fc = 2 * g + u
                                out = ps[:, ts(u, 512)]
                                for c in range(KC // 2):
                                    lhs_hi = Ehi_sb[:, pr(c), ts(dqt, P)]
                                    lhs_lo = Elo_sb[:, pr(c), ts(dqt, P)]
                                    rhs_hi = Vhi_sb[:, pr(c), ts(fc, 512)]
                                    rhs_lo = Vlo_sb[:, pr(c), ts(fc, 512)]
                                    nc.tensor.matmul(
                                        out, lhs_hi, rhs_hi, start=(c == 0),
                                        stop=False, perf_mode=DR)
                                    nc.tensor.matmul(
                                        out, lhs_hi, rhs_lo, start=False,
                                        stop=False, perf_mode=DR)
                                    nc.tensor.matmul(
                                        out, lhs_lo, rhs_hi, start=False,
                                        stop=(c == KC // 2 - 1), perf_mode=DR)
                            last = (dqt == DQ // P - 1 and g == 3)
                            if last:
                                # final tile: halves on both engines so the
                                # tail eviction+DMA chain is shortest
                                nc.scalar.activation(
                                    ot[:, g, 0:512], ps[:, 0:512],
                                    Copy, scale=1.0 / 32.0)
                                nc.vector.tensor_scalar(
                                    ot[:, g, 512:1024], ps[:, 512:1024],
                                    1.0 / 32.0, None, Mult)
                            elif (g % 2 == 0) if dqt < DQ // P - 1 \
                                    else (g % 2 == 1):
                                nc.scalar.activation(
                                    ot[:, g], ps[:], Copy, scale=1.0 / 32.0)
                            else:
                                nc.vector.tensor_scalar(
                                    ot[:, g], ps[:], 1.0 / 32.0, None, Mult)
                        if dqt < DQ // P - 1:
                            nc.sync.dma_start(
                                p_out[ts(dqt, P), :],
                                ot[:].rearrange("p u n -> p (u n)"))
                        else:
                            for g in range(3):
                                nc.sync.dma_start(
                                    p_out[ts(dqt, P), ts(g, 1024)], ot[:, g])
                            nc.sync.dma_start(
                                p_out[ts(dqt, P), 3072:3584], ot[:, 3, 0:512])
                            nc.sync.dma_start(
                                p_out[ts(dqt, P), 3584:4096], ot[:, 3, 512:1024])

    nc.compile()
    return nc


F8 = ml_dtypes.float8_e4m3


def _q8(a):
    return np.ascontiguousarray(np.asarray(a, dtype=np.float32).astype(F8))


def _split8(a):
    hi = _q8(a)
    lo = _q8(np.asarray(a, np.float32) - hi.astype(np.float32))
    return hi, lo


def _part_fold(a):
    """[R*128, N...] -> [128, R, N...]."""
    r = a.shape[0] // P
    return np.ascontiguousarray(
        a.reshape(r, P, *a.shape[1:]).transpose(1, 0, *range(2, a.ndim + 1))
    )


def _stile_pack(a):
    """[128, CT, S] -> [ST, 128, CT, 128]."""
    return np.ascontiguousarray(
        a.reshape(P, CT, ST, P).transpose(2, 0, 1, 3))


def make_in_maps(q, k, v, Wq, bq, Wk, bk, Wv, bv):
    """Host-side shard + layout + quantization prep."""
    f32 = np.float32
    q, k, v = (np.asarray(x, f32) for x in (q, k, v))
    Wq, Wk, Wv = (np.asarray(x, f32) for x in (Wq, Wk, Wv))
    bq, bk = np.asarray(bq, f32), np.asarray(bk, f32)

    kT8 = _stile_pack(_q8(_part_fold(np.ascontiguousarray(k.T))))
    kT8 = np.ascontiguousarray(  # group-pack: [ST//8, 128, 8, CT, 128]
        kT8.reshape(ST // 8, 8, P, CT, P).transpose(0, 2, 1, 3, 4))
    q8 = np.ascontiguousarray(  # [128(s), ST, DIN] at 1/2 scale
        _q8(q / 2).reshape(ST, P, DIN).transpose(1, 0, 2))
    wqhi, wqlo = _split8(_part_fold(np.ascontiguousarray(32 * Wq.T)))
    wvhi, wvlo = _split8(_part_fold(np.ascontiguousarray(32 * Wv.T)))
    b16 = _q8(16 * bq).reshape(1, 1, DQ)
    bq16 = np.ascontiguousarray(np.concatenate([b16, b16], axis=1))

    in_maps = []
    for i in range(NCORES):
        sl = slice(i * KSH, (i + 1) * KSH)
        wk_hi = _q8(_part_fold(np.ascontiguousarray(32 * Wk[sl].T)))
        vT_hi, vT_lo = _split8(_part_fold(np.ascontiguousarray(v[sl].T)))
        bk16 = _q8(16 * bk[sl]).reshape(1, 1, KSH)
        bk2 = np.ascontiguousarray(np.concatenate([bk16, bk16], axis=1))
        b32 = _q8(32 * bk[sl])
        bkb = np.ascontiguousarray(np.concatenate([b32, b32]))
        in_maps.append({
            "kT8": kT8, "q8": q8,
            "wkhi": wk_hi,
            "wqhi": wqhi, "wqlo": wqlo,
            "wvhi": wvhi, "wvlo": wvlo,
            "vThi": vT_hi, "vTlo": vT_lo,
            "bk2": bk2, "bkb": bkb, "bq16": bq16,
        })
    return in_maps


def combine(results, bv):
    """Host-side unshard: merge per-core partial sums."""
    P_tot = np.zeros((DQ, DK), np.float64)
    s_tot = np.zeros((DQ,), np.float64)
    for r in results:
        P_tot += r["p_out"].astype(np.float64)
        s_tot += r["s_out"].reshape(DQ).astype(np.float64)
    out = P_tot / s_tot[:, None] + np.asarray(bv, np.float64)[None, :]
    return out.astype(np.float32)


def get_nc():
    with _lock:
        if "nc" not in _cache:
            _cache["nc"] = _build_module()
        return _cache["nc"]


def _run_spmd(in_maps):
    from concourse._compat import axon_active
    from concourse import bass_utils

    nc = get_nc()
    if not axon_active():
        res = bass_utils.run_bass_kernel_spmd(nc, in_maps, list(range(NCORES)))
        return res.results
    r = _get_axon_runner(nc)
    return r.unpack(r.fn(*r.pack(in_maps)))


_SHARED = ("kT8", "q8", "wqhi", "wqlo", "wvhi", "wvlo", "bq16")


class _AxonRunner:
    def __init__(self, nc, donate):
        import jax
        import numpy as _np
        from jax.sharding import Mesh, PartitionSpec, NamedSharding
        from jax.experimental.shard_map import shard_map
        import concourse.mybir as mybir
        from concourse import bass2jax

        bass2jax.install_neuronx_cc_hook()
        pname = nc.partition_id_tensor.name if nc.partition_id_tensor else None

        self.in_names, self.out_names, out_avals, self.zero_outs = [], [], [], []
        for alloc in nc.m.functions[0].allocations:
            if not isinstance(alloc, mybir.MemoryLocationSet):
                continue
            name = alloc.memorylocations[0].name
            if alloc.kind == "ExternalInput":
                if name != pname:
                    self.in_names.append(name)
            elif alloc.kind == "ExternalOutput":
                shape = tuple(alloc.tensor_shape)
                dtype = mybir.dt.np(alloc.dtype)
                self.out_names.append(name)
                out_avals.append(jax.core.ShapedArray(shape, dtype))
                self.zero_outs.append(_np.zeros(shape, dtype))
        self.out_avals = out_avals
        n_params = len(self.in_names)
        n_outs = len(out_avals)
        all_in_names = list(self.in_names) + list(self.out_names)
        if pname is not None:
            all_in_names.append(pname)

        def _body(*args):
            operands = list(args)
            if pname is not None:
                operands.append(bass2jax.partition_id_tensor())
            outs = bass2jax._bass_exec_p.bind(
                *operands,
                out_avals=tuple(out_avals),
                in_names=tuple(all_in_names),
                out_names=tuple(self.out_names),
                lowering_input_output_aliases=(),
                sim_require_finite=True,
                sim_require_nnan=True,
                nc=nc,
            )
            return tuple(outs)

        devices = jax.devices()[:NCORES]
        self.mesh = Mesh(_np.asarray(devices), ("core",))
        rep, sh = PartitionSpec(), PartitionSpec("core")
        self.in_specs = tuple(
            rep if n in _SHARED else sh for n in self.in_names
        ) + (sh,) * n_outs
        out_specs = (sh,) * n_outs
        donate_argnums = (
            tuple(range(n_params, n_params + n_outs)) if donate else ()
        )
        self.fn = jax.jit(
            shard_map(_body, mesh=self.mesh, in_specs=self.in_specs,
                      out_specs=out_specs, check_rep=False),
            donate_argnums=donate_argnums, keep_unused=True,
        )
        self._jax = jax
        self._NamedSharding = NamedSharding

    def pack(self, in_maps):
        import numpy as _np
        args = []
        for name in self.in_names:
            if name in _SHARED:
                args.append(_np.asarray(in_maps[0][name]))
            else:
                args.append(
                    _np.concatenate(
                        [_np.asarray(m[name]) for m in in_maps], axis=0)
                )
        for z in self.zero_outs:
            args.append(_np.zeros((NCORES * z.shape[0], *z.shape[1:]), z.dtype))
        return args

    def to_device(self, args):
        return [
            self._jax.device_put(
                a, self._NamedSharding(self.mesh, spec))
            for a, spec in zip(args, self.in_specs)
        ]

    def unpack(self, out_arrs):
        import numpy as _np
        return [
            {
                name: _np.asarray(out_arrs[i]).reshape(
                    NCORES, *self.out_avals[i].shape)[c]
                for i, name in enumerate(self.out_names)
            }
            for c in range(NCORES)
        ]


def _get_axon_runner(nc, donate=False):
    key = ("runner", donate)
    with _lock:
        if key in _cache:
            return _cache[key]
    runner = _AxonRunner(nc, donate)
    with _lock:
        _cache[key] = runner
    return runner


def kernel(q, k, v, Wq, bq, Wk, bk, Wv, bv):
    q, k, v, Wq, bq, Wk, bk, Wv, bv = (
        np.asarray(a) for a in (q, k, v, Wq, bq, Wk, bk, Wv, bv))
    in_maps = make_in_maps(q, k, v, Wq, bq, Wk, bk, Wv, bv)
    results = _run_spmd(in_maps)
    return combine(results, np.asarray(bv))



# revision 34
# speedup vs baseline: 1.0343x; 1.0343x over previous
"""CrossAttentionHead kernel for 8x TRN2 NeuronCores (Bass/Tile), v2.

Reference (fp32):
    Q = q @ Wq.T + bq          # [S, DQ]      S=4096, DQ=1024
    K = k @ Wk.T + bk          # [S, DK]      DK=4096
    V = v @ Wv.T + bv          # [S, DK]
    numT = K.T @ Q             # [DK, DQ]
    attn = softmax(num / 64, axis over DK)
    out  = attn @ V            # [DQ, DK]

Sharding: tensor-parallel over dim_k; core i owns k-columns
[i*512, (i+1)*512).  Restructure kills the replicated Q projection:
    K_i  = k @ Wk_i^T + bk_i                 # [S, 512]
    A_i  = q^T K_i                           # [DIN, 512]  (contraction S)
    numT = A_i^T-as-lhsT @ Wq^T + colsum(K_i) x bq   # [512, DQ]
    E_i  = exp(numT / 64);  s_i = colsum(E_i)
    P_i  = E_i^T @ (v_i @ Wv^T)              # [DQ, DK] partial
Host merges: out = (sum P_i) / (sum s_i) + bv.

Precision: fp8e4m3 operands with DoubleRow matmuls (2 contraction
tiles / instruction).  Pure-fp8 tensors: k (1x), q (1/2 scale),
Wk (32x) and the on-chip K (32x).  Hi+lo split-fp8 (residual)
tensors: Wq,Wv (32x scale), V (32x), A (1/2), E (1x).  All psum
scales chosen so hi=copy(psum), lo=sub(psum,hi) need no rescaling
except A (tensor_scalar pass).
Scale bookkeeping:
    psum_K = 32K;  psum_A = 16A;  psum_num = 16*numT  (exp scale 1/1024)
    psum_V = 32V;  psum_P = 32P (evict fp16 w/ scale 1/32); psum_s = s
Validated vs reference data: maxrel 1.675e-2 (gate 2e-2).
"""

import threading

import numpy as np
import ml_dtypes

S = 4096
DIN = 1024
DQ = 1024
DK = 4096
NCORES = 8
KSH = DK // NCORES          # 512: per-core shard of dim_k
P = 128
ST = S // P                 # 32 s-tiles
CT = DIN // P               # 8 contraction chunks over DIN
KC = KSH // P               # 4 partition chunks of the k-shard
E_SCALE = 1.0 / 1024.0      # exp(psum_num * E_SCALE) = exp(numT/64)

_lock = threading.Lock()
_cache = {}


def _build_module():
    import concourse.bacc as bacc
    import concourse.mybir as mybir
    import concourse.tile as tile

    f8 = mybir.dt.float8e4
    f16 = mybir.dt.float16
    f32 = mybir.dt.float32
    Exp = mybir.ActivationFunctionType.Exp
    Copy = mybir.ActivationFunctionType.Copy
    DR = mybir.MatmulPerfMode.DoubleRow
    Mult = mybir.AluOpType.mult

    nc = bacc.Bacc(
        "TRN2", target_bir_lowering=False, debug=False, num_devices=NCORES
    )

    # DRAM inputs (per-core layouts prepped host-side)
    kT8g = nc.dram_tensor(
        "kT8", [ST // 8, P, 8, CT, P], f8, kind="ExternalInput").ap()
    q8 = nc.dram_tensor("q8", [P, ST, DIN], f8, kind="ExternalInput").ap()
    wkhi = nc.dram_tensor("wkhi", [P, CT, KSH], f8, kind="ExternalInput").ap()
    wqhi = nc.dram_tensor("wqhi", [P, CT, DQ], f8, kind="ExternalInput").ap()
    wqlo = nc.dram_tensor("wqlo", [P, CT, DQ], f8, kind="ExternalInput").ap()
    wvhi = nc.dram_tensor("wvhi", [P, CT, DK], f8, kind="ExternalInput").ap()
    wvlo = nc.dram_tensor("wvlo", [P, CT, DK], f8, kind="ExternalInput").ap()
    vThi = nc.dram_tensor("vThi", [P, CT, KSH], f8, kind="ExternalInput").ap()
    vTlo = nc.dram_tensor("vTlo", [P, CT, KSH], f8, kind="ExternalInput").ap()
    cs2 = nc.dram_tensor("cs2", [1, 2, KSH], f8, kind="ExternalInput").ap()
    bq16 = nc.dram_tensor(
        "bq16", [1, 2, DQ], f8, kind="ExternalInput").ap()
    p_out = nc.dram_tensor("p_out", [DQ, DK], f16, kind="ExternalOutput").ap()
    s_out = nc.dram_tensor("s_out", [1, DQ], f32, kind="ExternalOutput").ap()

    def ts(i, sz):
        return slice(i * sz, (i + 1) * sz)

    def pr(i):  # DoubleRow pair slice over a chunk axis
        return slice(2 * i, 2 * i + 2)

    with tile.TileContext(nc) as tc:
        with tc.tile_pool(name="persist", bufs=1) as persist:
            # Persistent SBUF tensors
            q_sb = persist.tile([P, ST, DIN], f8)       # 32 KB/part
            Khi_sb = persist.tile([P, ST, KSH], f8)     # 16 KB
            Ahi_sb = persist.tile([P, CT, KSH], f8)     # 4 KB
            Alo_sb = persist.tile([P, CT, KSH], f8)     # 4 KB
            Ef_sb = persist.tile([P, KC, DQ], f16)      # 8 KB
            Ehi_sb = persist.tile([P, KC, DQ], f8)      # 4 KB
            Elo_sb = persist.tile([P, KC, DQ], f8)      # 4 KB
            Vhi_sb = persist.tile([P, KC, DK], f8)      # 16 KB
            Vlo_sb = persist.tile([P, KC, DK], f8)      # 16 KB
            wkhi_sb = persist.tile([P, CT, KSH], f8)    # 4 KB
            wqhi_sb = persist.tile([P, CT, DQ], f8)     # 8 KB
            wqlo_sb = persist.tile([P, CT, DQ], f8)     # 8 KB
            vThi_sb = persist.tile([P, CT, KSH], f8)    # 4 KB
            vTlo_sb = persist.tile([P, CT, KSH], f8)    # 4 KB
            bq2_sb = persist.tile([1, 2, DQ], f8)
            cs2_sb = persist.tile([1, 2, KSH], f8)
            s_sb = persist.tile([1, DQ], f32)
            ones2c = persist.tile([P, 2, P], f8)        # DR ones, 128-part
            warm_out = persist.tile([P, P], f16)
            fin_sb = persist.tile([P, 512], f16)        # final DVE half
            # Pool memset: fast and keeps DVE free for real work
            nc.gpsimd.memset(ones2c[:], 1.0)

            # ---- input DMA (sync queue), ordered by first use ----
            # vT first: V-proj fc=0 is the cold-start filler phase
            nc.sync.dma_start(vThi_sb[:, 0:4], vThi[:, 0:4])

            wvp = tc.alloc_tile_pool(name="wv", bufs=4)
            ktp = tc.alloc_tile_pool(name="kt", bufs=3)
            # prefetch wv chunks fc=0,1 early (scalar/ACT queue)
            wv_tiles = {}
            whi0 = wvp.tile([P, CT, 512], f8, tag="wvhi")
            nc.sync.dma_start(whi0[:, 0:4], wvhi[:, 0:4, ts(0, 512)])
            nc.sync.dma_start(vThi_sb[:, 4:8], vThi[:, 4:8])
            nc.sync.dma_start(whi0[:, 4:8], wvhi[:, 4:8, ts(0, 512)])
            wlo0 = wvp.tile([P, CT, 512], f8, tag="wvlo")
            nc.sync.dma_start(wlo0[:], wvlo[:, :, ts(0, 512)])
            wv_tiles[0] = (whi0, wlo0)
            nc.sync.dma_start(vTlo_sb[:], vTlo)
            nc.sync.dma_start(cs2_sb[:], cs2)
            # V-proj fc=1 joins the cold start: its weights queue next, then
            # wk + the kt tiles, so Kproj's inputs land before Kproj starts
            whi1 = wvp.tile([P, CT, 512], f8, tag="wvhi")
            nc.sync.dma_start(whi1[:], wvhi[:, :, ts(1, 512)])
            wlo1 = wvp.tile([P, CT, 512], f8, tag="wvlo")
            nc.sync.dma_start(wlo1[:], wvlo[:, :, ts(1, 512)])
            wv_tiles[1] = (whi1, wlo1)
            nc.sync.dma_start(wkhi_sb[:, 0:2], wkhi[:, 0:2])

            # One shared PSUM pool: 4 slots x [P, 1024] f32 = all 8 banks.
            with tc.tile_pool(name="psum", bufs=4, space="PSUM") as pp:

                # PE warmup: the clock ramps over ~3us of continuous PE
                # execution, so start a cheap dummy chain ASAP (gated only
                # on the ones2c Pool memset) and keep it running until the
                # first input DMAs land (~3.4us).  N=128 keeps each dummy
                # matmul to ~53ns so the overrun past data-arrival is nil.
                wps = pp.tile([P, 1024], f32, tag="ps", name="warm")
                NWARM = 44
                for i in range(NWARM):
                    nc.tensor.matmul(
                        wps[:, 0:P], ones2c[0:1], ones2c[0:1],
                        start=(i == 0), stop=(i == NWARM - 1), perf_mode=DR,
                    )
                nc.scalar.activation(warm_out[:], wps[:, 0:P], Copy)

                def _vproj_fc(fc, prefetch=True):
                    if prefetch and fc + 3 < CT and fc + 3 not in wv_tiles:
                        whi3 = wvp.tile([P, CT, 512], f8, tag="wvhi")
                        nc.sync.dma_start(
                            whi3[:], wvhi[:, :, ts(fc + 3, 512)])
                        wlo3 = wvp.tile([P, CT, 512], f8, tag="wvlo")
                        nc.sync.dma_start(
                            wlo3[:], wvlo[:, :, ts(fc + 3, 512)])
                        wv_tiles[fc + 3] = (whi3, wlo3)
                    if prefetch and fc + 2 < CT and fc + 2 not in wv_tiles:
                        whi2 = wvp.tile([P, CT, 512], f8, tag="wvhi")
                        nc.sync.dma_start(
                            whi2[:], wvhi[:, :, ts(fc + 2, 512)])
                        wlo2 = wvp.tile([P, CT, 512], f8, tag="wvlo")
                        nc.sync.dma_start(
                            wlo2[:], wvlo[:, :, ts(fc + 2, 512)])
                        wv_tiles[fc + 2] = (whi2, wlo2)
                    whi, wlo = wv_tiles.pop(fc)
                    if fc <= 1:
                        # cold start: term-major, each term gated on a
                        # single DMA arrival
                        pss = [pp.tile([P, 1024], f32, tag="ps",
                                       name=f"psv{fc}_{c}") for c in range(2)]
                        for term in range(3):
                            for c in range(KC // 2):
                                for i in range(2):
                                    kc = 2 * c + i
                                    out = pss[c][:, ts(i, 512)]
                                    for dp in range(CT // 2):
                                        lhs = (vThi_sb if term < 2
                                               else vTlo_sb)[:, pr(dp), ts(kc, P)]
                                        rhs = (wlo if term == 1 else whi)[
                                            :, pr(dp), :]
                                        nc.tensor.matmul(
                                            out, lhs, rhs,
                                            start=(term == 0 and dp == 0),
                                            stop=(term == 2
                                                  and dp == CT // 2 - 1),
                                            perf_mode=DR)
                        for c in range(KC // 2):
                            psv = pss[c][:].rearrange("p (u n) -> p u n", u=2)
                            tgt_hi = Vhi_sb[:, pr(c), ts(fc, 512)]
                            nc.scalar.activation(tgt_hi, psv, Copy)
                            nc.vector.tensor_sub(
                                Vlo_sb[:, pr(c), ts(fc, 512)], psv, tgt_hi)
                        return
                    for c in range(KC // 2):
                        ps = pp.tile([P, 1024], f32, tag="ps")
                        for i in range(2):
                            kc = 2 * c + i
                            out = ps[:, ts(i, 512)]
                            for dp in range(CT // 2):
                                lhs_hi = vThi_sb[:, pr(dp), ts(kc, P)]
                                lhs_lo = vTlo_sb[:, pr(dp), ts(kc, P)]
                                nc.tensor.matmul(
                                    out, lhs_hi, whi[:, pr(dp), :],
                                    start=(dp == 0), stop=False, perf_mode=DR)
                                nc.tensor.matmul(
                                    out, lhs_hi, wlo[:, pr(dp), :],
                                    start=False, stop=False, perf_mode=DR)
                                nc.tensor.matmul(
                                    out, lhs_lo, whi[:, pr(dp), :],
                                    start=False, stop=(dp == CT // 2 - 1),
                                    perf_mode=DR)
                        psv = ps[:].rearrange("p (u n) -> p u n", u=2)
                        tgt_hi = Vhi_sb[:, pr(c), ts(fc, 512)]
                        nc.scalar.activation(tgt_hi, psv, Copy)
                        nc.vector.tensor_sub(
                            Vlo_sb[:, pr(c), ts(fc, 512)], psv, tgt_hi)

                # ---- phase 0: V proj fc=0,1 (cold-start fillers) ----
                _vproj_fc(0, prefetch=False)
                _vproj_fc(1, prefetch=False)

                # ---- phase 1: K proj (psum = 32K, 2 s-tiles per slot) ----
                kts = [None] * (ST // 8)

                def _fetch_kt(j):
                    kt = ktp.tile([P, 8, CT, P], f8, tag="kt")
                    if j == 0:
                        # interleave wk chunks with the first st tiles, in
                        # the dp-major order group 0 consumes them
                        nc.sync.dma_start(kt[:, 0:2], kT8g[0][:, 0:2])
                        nc.sync.dma_start(wkhi_sb[:, 2:4], wkhi[:, 2:4])
                        nc.sync.dma_start(kt[:, 2:4], kT8g[0][:, 2:4])
                        nc.sync.dma_start(wkhi_sb[:, 4:6], wkhi[:, 4:6])
                        nc.sync.dma_start(kt[:, 4:6], kT8g[0][:, 4:6])
                        nc.sync.dma_start(wkhi_sb[:, 6:8], wkhi[:, 6:8])
                        nc.sync.dma_start(kt[:, 6:8], kT8g[0][:, 6:8])
                    else:
                        nc.sync.dma_start(kt[:], kT8g[j])
                    kts[j] = kt

                _fetch_kt(0)
                _fetch_kt(1)

                # group 0 runs dp-major: each (dp, hi/lo) is an 8-st burst
                # needing only one fresh wk chunk, so the cold-start DMA
                # latencies overlap compute instead of serializing
                kt = kts[0]
                ps0 = [pp.tile([P, 1024], f32, tag="ps", name=f"ps0_{g}")
                       for g in range(4)]
                for dp in range(CT // 2):
                    for st in range(8):
                        nc.tensor.matmul(
                            ps0[st // 2][:, ts(st % 2, KSH)],
                            kt[:, st, pr(dp), :], wkhi_sb[:, pr(dp), :],
                            start=(dp == 0),
                            stop=(dp == CT // 2 - 1), perf_mode=DR,
                        )
                for g in range(4):
                    # K is bias-free: bk folds into the num-phase rank-2
                    # correction (see cs2/bq16), so evictions are pure copies
                    psv = ps0[g][:].rearrange("p (u n) -> p u n", u=2)
                    if g % 2 == 0:
                        nc.scalar.activation(Khi_sb[:, ts(g, 2)], psv, Copy)
                    else:
                        nc.vector.tensor_copy(Khi_sb[:, ts(g, 2)], psv)
                _fetch_kt(2)

                for j in range(1, ST // 8):
                    if j + 2 < ST // 8:
                        _fetch_kt(j + 2)
                    nc.sync.dma_start(
                        q_sb[:, ts(2 * j - 2, 4)], q8[:, ts(2 * j - 2, 4)])
                    nc.sync.dma_start(
                        q_sb[:, ts(2 * j - 1, 4)], q8[:, ts(2 * j - 1, 4)])
                    kt = kts[j]
                    for g in range(4):
                        ps = pp.tile([P, 1024], f32, tag="ps")
                        for u in range(2):
                            st = 8 * j + 2 * g + u
                            out = ps[:, ts(u, KSH)]
                            lt = kt[:, 2 * g + u]
                            for dp in range(CT // 2):
                                nc.tensor.matmul(
                                    out, lt[:, pr(dp), :], wkhi_sb[:, pr(dp), :],
                                    start=(dp == 0),
                                    stop=(dp == CT // 2 - 1),
                                    perf_mode=DR,
                                )
                        sl2 = ts(4 * j + g, 2)
                        psv = ps[:].rearrange("p (u n) -> p u n", u=2)
                        if g % 2 == 0:
                            nc.scalar.activation(Khi_sb[:, sl2], psv, Copy)
                        else:
                            nc.vector.tensor_copy(Khi_sb[:, sl2], psv)
                for c in range(6, 8):
                    nc.sync.dma_start(q_sb[:, ts(c, 4)], q8[:, ts(c, 4)])
                ktp.release()
                atp = tc.alloc_tile_pool(name="at", bufs=1)

                # ---- phase 2: A = q^T K (psum = 16A), 2 passes x 4 dc ----
                for pas in range(2):
                    pss = [pp.tile([P, 1024], f32, tag="ps",
                                   name=f"psa{pas}_{h}") for h in range(2)]
                    for t in range(ST // 2):
                        for i in range(4):
                            dc = 4 * pas + i
                            out = pss[i // 2][:, ts(i % 2, KSH)]
                            lhs = q_sb[:, pr(t), ts(dc, P)]
                            nc.tensor.matmul(
                                out, lhs, Khi_sb[:, pr(t), :],
                                start=(t == 0), stop=(t == ST // 2 - 1),
                                perf_mode=DR,
                            )
                    for h in range(2):
                        sl2 = ts(2 * pas + h, 2)
                        psv = pss[h][:].rearrange("p (u n) -> p u n", u=2)
                        nc.scalar.activation(
                            Ahi_sb[:, sl2], psv, Copy, scale=1.0 / 32.0)
                        at = atp.tile([P, 1024], f16, tag="at")
                        atv = at[:].rearrange("p (u n) -> p u n", u=2)
                        nc.vector.tensor_scalar(
                            atv, psv, 1.0 / 32.0, None, Mult)
                        nc.vector.tensor_sub(
                            Alo_sb[:, sl2], atv, Ahi_sb[:, sl2])
                atp.release()

                # (cs = colsum(K) comes in via the cs2 input: the host
                # computes colsum(k) @ Wk_i^T + S*bk_i exactly)

                # wv fc=2 + wq for num + fc=3,4 prefetch: the transfers ride
                # the DMA window left open once q has landed mid-A
                _whi = wvp.tile([P, CT, 512], f8, tag="wvhi")
                nc.sync.dma_start(_whi[:], wvhi[:, :, ts(2, 512)])
                _wlo = wvp.tile([P, CT, 512], f8, tag="wvlo")
                nc.sync.dma_start(_wlo[:], wvlo[:, :, ts(2, 512)])
                wv_tiles[2] = (_whi, _wlo)
                nc.sync.dma_start(bq2_sb[:], bq16)
                nc.sync.dma_start(wqhi_sb[:], wqhi)
                nc.sync.dma_start(wqlo_sb[:], wqlo)
                for _pf in (3, 4):
                    _whi = wvp.tile([P, CT, 512], f8, tag="wvhi")
                    nc.sync.dma_start(_whi[:], wvhi[:, :, ts(_pf, 512)])
                    _wlo = wvp.tile([P, CT, 512], f8, tag="wvlo")
                    nc.sync.dma_start(_wlo[:], wvlo[:, :, ts(_pf, 512)])
                    wv_tiles[_pf] = (_whi, _wlo)

                # ---- phase 4a: V proj fc=2 covers the A-evict latency ----
                _vproj_fc(2, prefetch=False)

                # ---- phase 4: numT (psum = 16*numT) + exp -> E hi/lo ----
                for kc in range(KC):
                    ps = pp.tile([P, DQ], f32, tag="ps", name=f"psn{kc}")
                    for h in range(2):
                        out = ps[:, ts(h, 512)]
                        for dp in range(CT // 2):
                            lhs_hi = Ahi_sb[:, pr(dp), ts(kc, P)]
                            lhs_lo = Alo_sb[:, pr(dp), ts(kc, P)]
                            rhs_hi = wqhi_sb[:, pr(dp), ts(h, 512)]
                            rhs_lo = wqlo_sb[:, pr(dp), ts(h, 512)]
                            nc.tensor.matmul(
                                out, lhs_hi, rhs_hi, start=(dp == 0),
                                stop=False, perf_mode=DR)
                            nc.tensor.matmul(
                                out, lhs_hi, rhs_lo, start=False,
                                stop=False, perf_mode=DR)
                            nc.tensor.matmul(
                                out, lhs_lo, rhs_hi, start=False,
                                stop=False, perf_mode=DR)
                        # + 2 x (cs/2) x (16bq) = 16 * cs x bq  (DoubleRow)
                        nc.tensor.matmul(
                            out, cs2_sb[:, :, ts(kc, P)],
                            bq2_sb[:, :, ts(h, 512)],
                            start=False, stop=True, perf_mode=DR,
                        )
                    nc.scalar.activation(
                        Ef_sb[:, kc], ps[:], Exp, scale=E_SCALE)
                    nc.vector.tensor_copy(Ehi_sb[:, kc], Ef_sb[:, kc])
                    nc.vector.tensor_sub(
                        Elo_sb[:, kc], Ef_sb[:, kc], Ehi_sb[:, kc])

                # ---- phase 5/6: remaining V proj + s denominators ----
                for fc in range(3, CT):
                    _vproj_fc(fc)
                    if fc == 3:
                        # s = colsum(E); E is ready by now, PE otherwise busy
                        for h in range(2):
                            ps = pp.tile([P, 512], f32, tag="ps", name=f"pss{h}")
                            for c in range(KC // 2):
                                nc.tensor.matmul(
                                    ps[:], ones2c[:],
                                    Ehi_sb[:, pr(c), ts(h, 512)],
                                    start=(c == 0), stop=False, perf_mode=DR,
                                )
                                nc.tensor.matmul(
                                    ps[:], ones2c[:],
                                    Elo_sb[:, pr(c), ts(h, 512)],
                                    start=False, stop=(c == KC // 2 - 1),
                                    perf_mode=DR,
                                )
                            nc.vector.tensor_copy(s_sb[:, ts(h, 512)], ps[0:1, :])
                        nc.sync.dma_start(s_out, s_sb[:])
                wvp.release()

                # ---- phase 7: P = E^T V (psum = 32P -> fp16 out) ----
                with tc.tile_pool(name="ost", bufs=2) as ost:
                    NR = DQ // P
                    for dqt in range(NR):
                        ot = ost.tile([P, 4, 1024], f16, tag="ot")
                        row = ts(dqt, P)
                        for g in range(4):  # fc pairs
                            ps = pp.tile([P, 1024], f32, tag="ps")
                            for u in range(2):
                                fc = 2 * g + u
                                out = ps[:, ts(u, 512)]
                                for c in range(KC // 2):
                                    lhs_hi = Ehi_sb[:, pr(c), ts(dqt, P)]
                                    lhs_lo = Elo_sb[:, pr(c), ts(dqt, P)]
                                    rhs_hi = Vhi_sb[:, pr(c), ts(fc, 512)]
                                    rhs_lo = Vlo_sb[:, pr(c), ts(fc, 512)]
                                    nc.tensor.matmul(
                                        out, lhs_hi, rhs_hi, start=(c == 0),
                                        stop=False, perf_mode=DR)
                                    nc.tensor.matmul(
                                        out, lhs_hi, rhs_lo, start=False,
                                        stop=False, perf_mode=DR)
                                    nc.tensor.matmul(
                                        out, lhs_lo, rhs_hi, start=False,
                                        stop=(c == KC // 2 - 1), perf_mode=DR)
                            last = (dqt == NR - 1 and g == 3)
                            if last:
                                # final tile: halves on both engines and
                                # DISJOINT tiles (whole-tile dep tracking
                                # would serialize two writers of ot)
                                nc.scalar.activation(
                                    ot[:, g, 0:512], ps[:, 0:512],
                                    Copy, scale=1.0 / 32.0)
                                nc.vector.tensor_scalar(
                                    fin_sb[:], ps[:, 512:1024],
                                    1.0 / 32.0, None, Mult)
                                nc.sync.dma_start(
                                    p_out[row, 3072:3584], ot[:, 3, 0:512])
                                nc.sync.dma_start(
                                    p_out[row, 3584:4096], fin_sb[:])
                            else:
                                if (g % 2 == 0) if dqt < NR - 1 \
                                        else (g % 2 == 1):
                                    nc.scalar.activation(
                                        ot[:, g], ps[:], Copy,
                                        scale=1.0 / 32.0)
                                else:
                                    nc.vector.tensor_scalar(
                                        ot[:, g], ps[:], 1.0 / 32.0,
                                        None, Mult)
                                if dqt == NR - 1:
                                    # last row: post per-g right after its
                                    # eviction so only 128KB remains at end
                                    nc.sync.dma_start(
                                        p_out[row, ts(g, 1024)], ot[:, g])
                        if dqt < NR - 2:
                            nc.sync.dma_start(
                                p_out[row, :],
                                ot[:].rearrange("p u n -> p (u n)"))
                        elif dqt == NR - 2:
                            # split: the 2nd-to-last row must not hog the
                            # DMA engine while the last row's chunks queue
                            nc.sync.dma_start(
                                p_out[row, 0:2048],
                                ot[:, 0:2].rearrange("p u n -> p (u n)"))
                            nc.sync.dma_start(
                                p_out[row, 2048:4096],
                                ot[:, 2:4].rearrange("p u n -> p (u n)"))

    nc.compile()
    return nc


F8 = ml_dtypes.float8_e4m3


def _q8(a):
    return np.ascontiguousarray(np.asarray(a, dtype=np.float32).astype(F8))


def _split8(a):
    hi = _q8(a)
    lo = _q8(np.asarray(a, np.float32) - hi.astype(np.float32))
    return hi, lo


def _part_fold(a):
    """[R*128, N...] -> [128, R, N...]."""
    r = a.shape[0] // P
    return np.ascontiguousarray(
        a.reshape(r, P, *a.shape[1:]).transpose(1, 0, *range(2, a.ndim + 1))
    )


def _stile_pack(a):
    """[128, CT, S] -> [ST, 128, CT, 128]."""
    return np.ascontiguousarray(
        a.reshape(P, CT, ST, P).transpose(2, 0, 1, 3))


def make_in_maps(q, k, v, Wq, bq, Wk, bk, Wv, bv):
    """Host-side shard + layout + quantization prep."""
    f32 = np.float32
    q, k, v = (np.asarray(x, f32) for x in (q, k, v))
    Wq, Wk, Wv = (np.asarray(x, f32) for x in (Wq, Wk, Wv))
    bq, bk = np.asarray(bq, f32), np.asarray(bk, f32)

    kT8 = _stile_pack(_q8(_part_fold(np.ascontiguousarray(k.T))))
    kT8 = np.ascontiguousarray(  # group-pack: [ST//8, 128, 8, CT, 128]
        kT8.reshape(ST // 8, 8, P, CT, P).transpose(0, 2, 1, 3, 4))
    q8 = np.ascontiguousarray(  # [128(s), ST, DIN] at 1/2 scale
        _q8(q / 2).reshape(ST, P, DIN).transpose(1, 0, 2))
    wqhi, wqlo = _split8(_part_fold(np.ascontiguousarray(32 * Wq.T)))
    wvhi, wvlo = _split8(_part_fold(np.ascontiguousarray(32 * Wv.T)))
    # num-phase rank-2 bias correction (K kept bias-free on device):
    #   numT += cs0 x bq + bk x (Qs + S*bq)
    # realized by the DR matmul  cs2(2 rows) x bq16(2 rows) at 16x psum:
    #   row0: cs0 * 16bq ; row1: 32bk * (Qs + S*bq)/2
    ksum = k.sum(0, dtype=np.float64)              # colsum(k), exact
    qsum = q.sum(0, dtype=np.float64)              # colsum(q), exact
    Qs = qsum @ Wq.T.astype(np.float64) + S * bq.astype(np.float64)
    b16 = _q8(16 * bq).reshape(1, 1, DQ)
    qs2 = _q8(Qs.astype(np.float32) / 2).reshape(1, 1, DQ)
    bq16 = np.ascontiguousarray(np.concatenate([b16, qs2], axis=1))

    in_maps = []
    for i in range(NCORES):
        sl = slice(i * KSH, (i + 1) * KSH)
        wk_hi = _q8(_part_fold(np.ascontiguousarray(32 * Wk[sl].T)))
        vT_hi, vT_lo = _split8(_part_fold(np.ascontiguousarray(v[sl].T)))
        cs0 = (ksum @ Wk[sl].T.astype(np.float64)).astype(np.float32)
        c8 = _q8(cs0).reshape(1, 1, KSH)
        b32 = _q8(32 * bk[sl]).reshape(1, 1, KSH)
        cs2 = np.ascontiguousarray(np.concatenate([c8, b32], axis=1))
        in_maps.append({
            "kT8": kT8, "q8": q8,
            "wkhi": wk_hi,
            "wqhi": wqhi, "wqlo": wqlo,
            "wvhi": wvhi, "wvlo": wvlo,
            "vThi": vT_hi, "vTlo": vT_lo,
            "cs2": cs2, "bq16": bq16,
        })
    return in_maps


def combine(results, bv):
    """Host-side unshard: merge per-core partial sums."""
    P_tot = np.zeros((DQ, DK), np.float64)
    s_tot = np.zeros((DQ,), np.float64)
    for r in results:
        P_tot += r["p_out"].astype(np.float64)
        s_tot += r["s_out"].reshape(DQ).astype(np.float64)
    out = P_tot / s_tot[:, None] + np.asarray(bv, np.float64)[None, :]
    return out.astype(np.float32)


def get_nc():
    with _lock:
        if "nc" not in _cache:
            _cache["nc"] = _build_module()
        return _cache["nc"]


def _run_spmd(in_maps):
    from concourse._compat import axon_active
    from concourse import bass_utils

    nc = get_nc()
    if not axon_active():
        res = bass_utils.run_bass_kernel_spmd(nc, in_maps, list(range(NCORES)))
        return res.results
    r = _get_axon_runner(nc)
    return r.unpack(r.fn(*r.pack(in_maps)))


_SHARED = ("kT8", "q8", "wqhi", "wqlo", "wvhi", "wvlo", "bq16")


class _AxonRunner:
    def __init__(self, nc, donate):
        import jax
        import numpy as _np
        from jax.sharding import Mesh, PartitionSpec, NamedSharding
        from jax.experimental.shard_map import shard_map
        import concourse.mybir as mybir
        from concourse import bass2jax

        bass2jax.install_neuronx_cc_hook()
        pname = nc.partition_id_tensor.name if nc.partition_id_tensor else None

        self.in_names, self.out_names, out_avals, self.zero_outs = [], [], [], []
        for alloc in nc.m.functions[0].allocations:
            if not isinstance(alloc, mybir.MemoryLocationSet):
                continue
            name = alloc.memorylocations[0].name
            if alloc.kind == "ExternalInput":
                if name != pname:
                    self.in_names.append(name)
            elif alloc.kind == "ExternalOutput":
                shape = tuple(alloc.tensor_shape)
                dtype = mybir.dt.np(alloc.dtype)
                self.out_names.append(name)
                out_avals.append(jax.core.ShapedArray(shape, dtype))
                self.zero_outs.append(_np.zeros(shape, dtype))
        self.out_avals = out_avals
        n_params = len(self.in_names)
        n_outs = len(out_avals)
        all_in_names = list(self.in_names) + list(self.out_names)
        if pname is not None:
            all_in_names.append(pname)

        def _body(*args):
            operands = list(args)
            if pname is not None:
                operands.append(bass2jax.partition_id_tensor())
            outs = bass2jax._bass_exec_p.bind(
                *operands,
                out_avals=tuple(out_avals),
                in_names=tuple(all_in_names),
                out_names=tuple(self.out_names),
                lowering_input_output_aliases=(),
                sim_require_finite=True,
                sim_require_nnan=True,
                nc=nc,
            )
            return tuple(outs)

        devices = jax.devices()[:NCORES]
        self.mesh = Mesh(_np.asarray(devices), ("core",))
        rep, sh = PartitionSpec(), PartitionSpec("core")
        self.in_specs = tuple(
            rep if n in _SHARED else sh for n in self.in_names
        ) + (sh,) * n_outs
        out_specs = (sh,) * n_outs
        donate_argnums = (
            tuple(range(n_params, n_params + n_outs)) if donate else ()
        )
        self.fn = jax.jit(
            shard_map(_body, mesh=self.mesh, in_specs=self.in_specs,
                      out_specs=out_specs, check_rep=False),
            donate_argnums=donate_argnums, keep_unused=True,
        )
        self._jax = jax
        self._NamedSharding = NamedSharding

    def pack(self, in_maps):
        import numpy as _np
        args = []
        for name in self.in_names:
            if name in _SHARED:
                args.append(_np.asarray(in_maps[0][name]))
            else:
                args.append(
                    _np.concatenate(
                        [_np.asarray(m[name]) for m in in_maps], axis=0)
                )
        for z in self.zero_outs:
            args.append(_np.zeros((NCORES * z.shape[0], *z.shape[1:]), z.dtype))
        return args

    def to_device(self, args):
        return [
            self._jax.device_put(
                a, self._NamedSharding(self.mesh, spec))
            for a, spec in zip(args, self.in_specs)
        ]

    def unpack(self, out_arrs):
        import numpy as _np
        return [
            {
                name: _np.asarray(out_arrs[i]).reshape(
                    NCORES, *self.out_avals[i].shape)[c]
                for i, name in enumerate(self.out_names)
            }
            for c in range(NCORES)
        ]


def _get_axon_runner(nc, donate=False):
    key = ("runner", donate)
    with _lock:
        if key in _cache:
            return _cache[key]
    runner = _AxonRunner(nc, donate)
    with _lock:
        _cache[key] = runner
    return runner


def kernel(q, k, v, Wq, bq, Wk, bk, Wv, bv):
    q, k, v, Wq, bq, Wk, bk, Wv, bv = (
        np.asarray(a) for a in (q, k, v, Wq, bq, Wk, bk, Wv, bv))
    in_maps = make_in_maps(q, k, v, Wq, bq, Wk, bk, Wv, bv)
    results = _run_spmd(in_maps)
    return combine(results, np.asarray(bv))

